# revision 28
# baseline (speedup 1.0000x reference)
"""EdgeConv (knn -> edge conv -> BN -> LeakyReLU -> max over k) on 8 NeuronCores.

Sharding: data-parallel over batch B=8, one sample per core. BN batch
statistics are all-reduced across the 8 cores on-device.

Math: with W = [W1 | W2] (acting on [nbr-ctr | ctr]), define
  u[m, :] = W1 @ x[:, m]          (projected neighbor part)
  v[n, :] = (W2 - W1) @ x[:, n]   (projected center part)
then y[n, k, :] = u[idx[n, k], :] + v[n, :].  BN scale is monotone, so
  out[:, n] = LeakyReLU(a * (max_k u[idx[n,k]] + v[n]) + b)
with a = gamma*rsqrt(var+eps), b = beta - mean*a.  Stats (mean/var over
(B, N, K)) come from per-n fp16 sums of gathered u (and squares / v-cross
terms), reduced across rows by a PE ones-matmul and all-reduced over the
batch.

Selection (exact, fp32): per 512-column window take the top-16 via
max8 / match_replace / max8 + two max_index passes (capacity 16 >= the
max top-20 concentration per window), merge the 128 candidates with three
max8/match_replace rounds marking the global top-20 as NEG, then extract
the winners' global column indices with a mask*(32768-idx) compaction:
three max8 rounds over the masked-index image recover the 20 indices
without any per-winner mask-reduce scans.

Gather: u stored fp16 in DRAM (halving gather bytes); 20 per-k indirect
DMAs per row-tile (HW applies per-partition offsets only for [128, 1]
offset APs).  Max/sum/sumsq trees over the 20 gathered pages run in fp16
on DVE at 2x; squares come from the scalar engine; the final
affine+LeakyReLU runs as two scalar-engine affine passes plus one DVE max
over the transposed result.
"""
import sys
for p in ("/opt/trn_rl_repo", "/root/.axon_site/_ro/trn_rl_repo"):
    if p not in sys.path:
        sys.path.insert(0, p)

import numpy as np

B, C, N, O, K = 8, 64, 4096, 64, 20
EPS = 1e-5
ALPHA = 0.2
T = N // 128          # 32 row-tiles
SEG = 512             # psum eviction chunk (one bank)
NW = N // SEG
WSEL = 512            # selection window: top-16 each (max conc. 10)
NWS = N // WSEL       # 8 windows -> 128 cands/row
NCAND = NWS * 16
QBASE = 32768.0
NEG = -3.0e38

_CACHED = {}


def _build(num_devices=8):
    import concourse.bass as bass
    import concourse.bacc as bacc
    import concourse.mybir as mybir
    from concourse.tile import TileContext

    F32 = mybir.dt.float32
    F16 = mybir.dt.float16
    U16 = mybir.dt.uint16
    U32 = mybir.dt.uint32
    AF = mybir.ActivationFunctionType
    ALU = mybir.AluOpType

    nc = bacc.Bacc("TRN2", target_bir_lowering=False, num_devices=num_devices)

    x_d = nc.dram_tensor("x", [C, N], F32, kind="ExternalInput")
    w1t_d = nc.dram_tensor("w1t", [C, O], F32, kind="ExternalInput")
    w2t_d = nc.dram_tensor("w2t", [C, O], F32, kind="ExternalInput")
    gam_d = nc.dram_tensor("gamma", [1, O], F32, kind="ExternalInput")
    bet_d = nc.dram_tensor("beta", [1, O], F32, kind="ExternalInput")
    id_d = nc.dram_tensor("ident", [128, 128], F32, kind="ExternalInput")
    out_d = nc.dram_tensor("out", [O, N], F32, kind="ExternalOutput")

    u_d = nc.dram_tensor("u_scratch", [N, O], F16)
    cc_in = nc.dram_tensor("cc_in", [5 * O], F32)
    cc_out = nc.dram_tensor("cc_out", [5 * O], F32)
    ab_d = nc.dram_tensor("ab_scratch", [2 * O], F32)

    CNT = float(B * N * K)

    with TileContext(nc) as tc:
        with tc.tile_pool(name="big", bufs=1) as big, \
             tc.tile_pool(name="sc", bufs=2) as sc, \
             tc.tile_pool(name="ssb", bufs=2) as ssb, \
             tc.tile_pool(name="gpool", bufs=3) as gpool, \
             tc.tile_pool(name="chup", bufs=3) as chup, \
             tc.tile_pool(name="ps", bufs=2, space="PSUM") as ps, \
             tc.tile_pool(name="pt", bufs=2, space="PSUM") as pt, \
             tc.tile_pool(name="pstat", bufs=1, space="PSUM") as pstat:

            # ---------------- phase 0: prep ----------------
            zmv = big.tile([C + 1, N], F32)      # moving: [x; -xx]
            zst = big.tile([C + 1, N], F32)      # stationary: [2x; ones]
            for cs in range(NW):
                nc.sync.dma_start(zmv[0:C, SEG * cs:SEG * (cs + 1)],
                                  x_d[:, SEG * cs:SEG * (cs + 1)])
            id_sb = big.tile([128, 128], F32)
            nc.sync.dma_start(id_sb[:], id_d[:, :])
            w1_sb = big.tile([C, O], F32)
            nc.sync.dma_start(w1_sb[:], w1t_d[:, :])
            w2_sb = big.tile([C, O], F32)
            nc.sync.dma_start(w2_sb[:], w2t_d[:, :])
            g_sb = big.tile([1, O], F32)
            nc.sync.dma_start(g_sb[:], gam_d[:, :])
            be_sb = big.tile([1, O], F32)
            nc.sync.dma_start(be_sb[:], bet_d[:, :])

            wv_sb = big.tile([C, O], F32)
            nc.vector.tensor_sub(wv_sb[:], w2_sb[:], w1_sb[:])

            for cs in range(NW):
                nc.vector.tensor_scalar(out=zst[0:C, SEG * cs:SEG * (cs + 1)],
                                        in0=zmv[0:C, SEG * cs:SEG * (cs + 1)],
                                        scalar1=2.0, scalar2=None, op0=ALU.mult)
            nc.vector.memset(zst[C:C + 1, :], 1.0)

            ones64 = big.tile([C, 1], F32)
            nc.vector.memset(ones64[:], 1.0)
            ones128 = big.tile([128, 1], F16)
            nc.vector.memset(ones128[:], 1.0)
            for cs in range(NW):
                sqx = sc.tile([C, SEG], F32, tag="sqx")
                nc.vector.tensor_mul(sqx[:], zmv[0:C, SEG * cs:SEG * (cs + 1)],
                                     zmv[0:C, SEG * cs:SEG * (cs + 1)])
                xx_ps = ps.tile([1, SEG], F32, tag="s")
                nc.tensor.matmul(xx_ps[:], ones64[:], sqx[:],
                                 start=True, stop=True)
                nc.scalar.activation(out=zmv[C:C + 1, SEG * cs:SEG * (cs + 1)],
                                     in_=xx_ps[:], func=AF.Copy, scale=-1.0)

            u16_sb = big.tile([128, T, O], F16)
            v32_sb = big.tile([128, T, O], F32)
            v16_sb = big.tile([128, T, O], F16)

            def emit_uv():
                # emitted after sel(0) so the 64 projection matmuls and the
                # u-table DRAM write hide behind the first selection pass
                for t in range(T):
                    up = ps.tile([128, O], F32, tag="uv")
                    nc.tensor.matmul(up[:], zmv[0:C, 128 * t:128 * (t + 1)],
                                     w1_sb[:], start=True, stop=True)
                    nc.scalar.activation(out=u16_sb[:, t, :], in_=up[:],
                                         func=AF.Copy)
                    vp = ps.tile([128, O], F32, tag="uv")
                    nc.tensor.matmul(vp[:], zmv[0:C, 128 * t:128 * (t + 1)],
                                     wv_sb[:], start=True, stop=True)
                    nc.scalar.activation(out=v32_sb[:, t, :], in_=vp[:],
                                         func=AF.Copy)
                    nc.scalar.activation(out=v16_sb[:, t, :], in_=vp[:],
                                         func=AF.Copy)
                nc.sync.dma_start(u_d.ap().rearrange("(t p) o -> p t o", p=128),
                                  u16_sb[:])

            # window-start offsets per candidate slot: col c -> (c // 16) * WSEL
            offs_c = big.tile([128, NCAND], F32)
            for w in range(NWS):
                nc.vector.memset(offs_c[:, 16 * w:16 * (w + 1)], float(WSEL * w))
            mvp = big.tile([128, 8], F32)        # partial-round scratch
            nc.vector.memset(mvp[:], NEG)
            pcol_u = big.tile([128, 1], U32)     # partition index column
            nc.gpsimd.iota(pcol_u[:], pattern=[[0, 1]], base=0,
                           channel_multiplier=1)
            pcol = big.tile([128, 1], F32)
            nc.vector.tensor_copy(pcol[:], pcol_u[:])
            selfq = big.tile([128, 1], F32)      # per-tile QBASE-(128t+p)

            mfull = big.tile([O, N], F32)        # pre-norm max+v, transposed
            stats_ps = pstat.tile([1, 5 * O], F32)

            # ---------------- phase 1: per row-tile, software-pipelined ----
            # DVE emission order: sel(0), sel(1), trees(0), sel(2), trees(1),
            # ... so the k-gather of tile t completes behind selection of tile
            # t+1 instead of stalling the vector engine (~13us/tile).
            def emit_sel(t):
                s_sb = ssb.tile([128, N], F32, tag="s")
                s2_sb = ssb.tile([128, N], F32, tag="s2")
                for w in range(NW):
                    sp = ps.tile([128, SEG], F32, tag="s")
                    nc.tensor.matmul(sp[:], zst[:, 128 * t:128 * (t + 1)],
                                     zmv[:, SEG * w:SEG * (w + 1)],
                                     start=True, stop=True)
                    nc.scalar.activation(out=s_sb[:, SEG * w:SEG * (w + 1)],
                                         in_=sp[:], func=AF.Copy)

                cand = sc.tile([128, NWS, 2, 8], F32, tag="cand")
                cloc = sc.tile([128, NWS, 2, 8], U16, tag="cloc")
                for w in range(NWS):
                    sw = s_sb[:, WSEL * w:WSEL * (w + 1)]
                    s2w = s2_sb[:, WSEL * w:WSEL * (w + 1)]
                    nc.vector.max(out=cand[:, w, 0, :], in_=sw)
                    nc.vector.match_replace(out=s2w,
                                            in_to_replace=cand[:, w, 0, :],
                                            in_values=sw, imm_value=NEG)
                    nc.vector.max(out=cand[:, w, 1, :], in_=s2w)
                    nc.vector.max_index(out=cloc[:, w, 0, :],
                                        in_max=cand[:, w, 0, :], in_values=sw)
                    nc.vector.max_index(out=cloc[:, w, 1, :],
                                        in_max=cand[:, w, 1, :], in_values=s2w)

                globf = sc.tile([128, NCAND], F32, tag="globf")
                nc.vector.tensor_copy(globf[:], cloc[:])
                nc.vector.tensor_add(globf[:], globf[:], offs_c[:])
                nc.vector.tensor_scalar(out=globf[:], in0=globf[:],
                                        scalar1=-1.0, scalar2=QBASE,
                                        op0=ALU.mult, op1=ALU.add)

                mv = sc.tile([128, 8], F32, tag="mv")
                for r in range(3):
                    nc.vector.max(out=mv[:], in_=cand[:])
                    if r < 2:
                        nc.vector.match_replace(out=cand[:], in_to_replace=mv[:],
                                                in_values=cand[:], imm_value=NEG)
                nc.vector.tensor_copy(mvp[:, 0:4], mv[:, 0:4])
                nc.vector.match_replace(out=cand[:], in_to_replace=mvp[:],
                                        in_values=cand[:], imm_value=NEG)

                q = sc.tile([128, NCAND], F32, tag="q")
                nc.vector.scalar_tensor_tensor(out=q[:], in0=cand[:], scalar=NEG,
                                               in1=globf[:], op0=ALU.is_equal,
                                               op1=ALU.mult)
                # self (always rank 1) is served from SBUF, not gathered:
                # zero its q entry so compaction yields the other 19 indices
                nc.vector.tensor_scalar(out=selfq[:], in0=pcol[:], scalar1=-1.0,
                                        scalar2=QBASE - 128.0 * t,
                                        op0=ALU.mult, op1=ALU.add)
                nc.vector.scalar_tensor_tensor(out=q[:], in0=q[:],
                                               scalar=selfq[:, 0:1], in1=q[:],
                                               op0=ALU.not_equal, op1=ALU.mult)
                qsel = sc.tile([128, 24], F32, tag="qsel")
                for r in range(3):
                    nc.vector.max(out=qsel[:, 8 * r:8 * (r + 1)], in_=q[:])
                    if r < 2:
                        nc.vector.match_replace(out=q[:],
                                                in_to_replace=qsel[:, 8 * r:8 * (r + 1)],
                                                in_values=q[:], imm_value=0.0)
                nc.vector.tensor_scalar(out=qsel[:], in0=qsel[:],
                                        scalar1=-1.0, scalar2=QBASE,
                                        op0=ALU.mult, op1=ALU.add)
                chu = chup.tile([128, 24], U32, tag="chu")
                nc.vector.tensor_copy(chu[:], qsel[:])
                return chu

            def emit_gather(chu):
                gat = gpool.tile([128, K, O], F16, tag="gat")
                for k in range(K - 1):
                    nc.gpsimd.indirect_dma_start(
                        out=gat[:, k, :], out_offset=None, in_=u_d[:],
                        in_offset=bass.IndirectOffsetOnAxis(
                            ap=chu[:, k:k + 1], axis=0))
                return gat

            def emit_trees(t, gat):
                nc.scalar.activation(out=gat[:, K - 1, :], in_=u16_sb[:, t, :],
                                     func=AF.Copy)
                gv = gat[:, :, 0:O]
                gsq = gpool.tile([128, K, O], F16, tag="gsq")
                nc.scalar.activation(out=gsq[:], in_=gv, func=AF.Square)

                statcat = sc.tile([128, 3 * O], F16, tag="statcat")
                tm10 = sc.tile([128, 10, O], F16, tag="tm10")
                nc.vector.tensor_tensor(out=tm10[:], in0=gat[:, 0:10, 0:O],
                                        in1=gat[:, 10:20, 0:O], op=ALU.max)
                tm5 = sc.tile([128, 5, O], F16, tag="tm5")
                nc.vector.tensor_tensor(out=tm5[:], in0=tm10[:, 0:5, :],
                                        in1=tm10[:, 5:10, :], op=ALU.max)
                tm2 = sc.tile([128, 2, O], F16, tag="tm2")
                nc.vector.tensor_tensor(out=tm2[:], in0=tm5[:, 0:2, :],
                                        in1=tm5[:, 2:4, :], op=ALU.max)
                tm1 = sc.tile([128, O], F16, tag="tm1")
                nc.vector.tensor_tensor(out=tm1[:], in0=tm2[:, 0, :],
                                        in1=tm2[:, 1, :], op=ALU.max)
                mx = sc.tile([128, O], F16, tag="mx")
                nc.vector.tensor_tensor(out=mx[:], in0=tm1[:], in1=tm5[:, 4, :],
                                        op=ALU.max)

                ts10 = sc.tile([128, 10, O], F16, tag="ts10")
                nc.vector.tensor_tensor(out=ts10[:], in0=gat[:, 0:10, 0:O],
                                        in1=gat[:, 10:20, 0:O], op=ALU.add)
                ts5 = sc.tile([128, 5, O], F16, tag="ts5")
                nc.vector.tensor_tensor(out=ts5[:], in0=ts10[:, 0:5, :],
                                        in1=ts10[:, 5:10, :], op=ALU.add)
                ts2 = sc.tile([128, 2, O], F16, tag="ts2")
                nc.vector.tensor_tensor(out=ts2[:], in0=ts5[:, 0:2, :],
                                        in1=ts5[:, 2:4, :], op=ALU.add)
                ts1 = sc.tile([128, O], F16, tag="ts1")
                nc.vector.tensor_tensor(out=ts1[:], in0=ts2[:, 0, :],
                                        in1=ts2[:, 1, :], op=ALU.add)
                su = statcat[:, 0 * O:1 * O]
                nc.vector.tensor_tensor(out=su, in0=ts1[:], in1=ts5[:, 4, :],
                                        op=ALU.add)

                qs10 = sc.tile([128, 10, O], F16, tag="qs10")
                nc.vector.tensor_tensor(out=qs10[:], in0=gsq[:, 0:10, :],
                                        in1=gsq[:, 10:20, :], op=ALU.add)
                qs5 = sc.tile([128, 5, O], F16, tag="qs5")
                nc.vector.tensor_tensor(out=qs5[:], in0=qs10[:, 0:5, :],
                                        in1=qs10[:, 5:10, :], op=ALU.add)
                qs2 = sc.tile([128, 2, O], F16, tag="qs2")
                nc.vector.tensor_tensor(out=qs2[:], in0=qs5[:, 0:2, :],
                                        in1=qs5[:, 2:4, :], op=ALU.add)
                qs1 = sc.tile([128, O], F16, tag="qs1")
                nc.vector.tensor_tensor(out=qs1[:], in0=qs2[:, 0, :],
                                        in1=qs2[:, 1, :], op=ALU.add)
                ssq = statcat[:, 1 * O:2 * O]
                nc.vector.tensor_tensor(out=ssq, in0=qs1[:], in1=qs5[:, 4, :],
                                        op=ALU.add)
                vsu = statcat[:, 2 * O:3 * O]
                nc.vector.tensor_mul(vsu, v16_sb[:, t, :], su)

                st, sp_ = (t == 0), (t == T - 1)
                nc.tensor.matmul(stats_ps[:, 0:3 * O], ones128[:], statcat[:],
                                 start=st, stop=sp_, skip_group_check=True)

                mxv = sc.tile([128, O], F32, tag="mxv")
                nc.vector.tensor_tensor(out=mxv[:], in0=mx[:],
                                        in1=v32_sb[:, t, :], op=ALU.add)
                mt_ps = pt.tile([O, 128], F32, tag="mt")
                nc.tensor.transpose(out=mt_ps[:], in_=mxv[:], identity=id_sb[:])
                nc.scalar.activation(out=mfull[:, 128 * t:128 * (t + 1)],
                                     in_=mt_ps[:], func=AF.Copy)

            # depth-3 pipeline: trees(t) runs after sel(t+2), hiding both the
            # u-table warmup and per-tile gather latency
            gats = {}
            for t in range(T):
                chu = emit_sel(t)
                if t == 0:
                    emit_uv()   # u table lands while sel(0) runs on DVE
                gats[t] = emit_gather(chu)
                if t >= 2:
                    emit_trees(t - 2, gats.pop(t - 2))
            emit_trees(T - 2, gats.pop(T - 2))
            emit_trees(T - 1, gats.pop(T - 1))

            # v sums (once): Sv, Sv2 into stats_ps[:, 192:320]
            vsq16 = big.tile([128, T, O], F16)
            nc.scalar.activation(out=vsq16[:], in_=v16_sb[:], func=AF.Square)
            for t in range(T):
                st, sp_ = (t == 0), (t == T - 1)
                nc.tensor.matmul(stats_ps[:, 3 * O:4 * O], ones128[:],
                                 v16_sb[:, t, :], start=st, stop=sp_,
                                 skip_group_check=True)
                nc.tensor.matmul(stats_ps[:, 4 * O:5 * O], ones128[:],
                                 vsq16[:, t, :], start=st, stop=sp_,
                                 skip_group_check=True)

            # ---------------- phase 2: stats allreduce + finalize ----------
            stats_sb = big.tile([1, 5 * O], F32)
            nc.scalar.activation(out=stats_sb[:], in_=stats_ps[:], func=AF.Copy)
            if num_devices > 1:
                nc.sync.dma_start(cc_in.ap().rearrange("(a b) -> a b", a=1),
                                  stats_sb[:])
                nc.gpsimd.collective_compute(
                    "AllReduce", mybir.AluOpType.add,
                    replica_groups=[list(range(num_devices))],
                    ins=[cc_in.ap().opt()], outs=[cc_out.ap().opt()])
                sall = big.tile([1, 5 * O], F32)
                nc.sync.dma_start(sall[:],
                                  cc_out.ap().rearrange("(a b) -> a b", a=1))
            else:
                sall = stats_sb

            Sg = sall[:, 0 * O:1 * O]
            Sq = sall[:, 1 * O:2 * O]
            Svsu = sall[:, 2 * O:3 * O]
            Sv = sall[:, 3 * O:4 * O]
            Sv2 = sall[:, 4 * O:5 * O]

            mean = big.tile([1, O], F32)
            # mean = (Sg + K*Sv)/CNT
            nc.vector.tensor_scalar(out=mean[:], in0=Sv[:], scalar1=float(K),
                                    scalar2=None, op0=ALU.mult)
            nc.vector.tensor_add(mean[:], mean[:], Sg[:])
            nc.vector.tensor_scalar(out=mean[:], in0=mean[:],
                                    scalar1=1.0 / CNT, scalar2=None,
                                    op0=ALU.mult)
            ey2 = big.tile([1, O], F32)
            # ey2 = (Sq + 2*Svsu + K*Sv2)/CNT
            nc.vector.tensor_scalar(out=ey2[:], in0=Svsu[:], scalar1=2.0,
                                    scalar2=None, op0=ALU.mult)
            nc.vector.tensor_add(ey2[:], ey2[:], Sq[:])
            tmp = big.tile([1, O], F32)
            nc.vector.tensor_scalar(out=tmp[:], in0=Sv2[:], scalar1=float(K),
                                    scalar2=None, op0=ALU.mult)
            nc.vector.tensor_add(ey2[:], ey2[:], tmp[:])
            nc.vector.tensor_scalar(out=ey2[:], in0=ey2[:], scalar1=1.0 / CNT,
                                    scalar2=None, op0=ALU.mult)
            var = big.tile([1, O], F32)
            nc.vector.tensor_mul(var[:], mean[:], mean[:])
            nc.vector.tensor_sub(var[:], ey2[:], var[:])
            # rstd = 1/sqrt(var+eps) with two Newton-Raphson refinements
            std = big.tile([1, O], F32)
            epsb = big.tile([1, 1], F32)
            nc.vector.memset(epsb[:], EPS)
            ve = big.tile([1, O], F32)
            nc.vector.tensor_scalar(out=ve[:], in0=var[:], scalar1=EPS,
                                    scalar2=None, op0=ALU.add)
            nc.scalar.activation(out=std[:], in_=var[:], func=AF.Sqrt,
                                 bias=epsb[:], scale=1.0)
            rstd = big.tile([1, O], F32)
            nc.vector.reciprocal(rstd[:], std[:])
            nr1 = big.tile([1, O], F32)
            nr2 = big.tile([1, O], F32)
            for _ in range(2):
                nc.vector.tensor_mul(nr1[:], rstd[:], rstd[:])
                nc.vector.tensor_mul(nr1[:], nr1[:], ve[:])
                nc.vector.tensor_scalar(out=nr2[:], in0=nr1[:], scalar1=-0.5,
                                        scalar2=1.5, op0=ALU.mult, op1=ALU.add)
                nc.vector.tensor_mul(rstd[:], rstd[:], nr2[:])

            ab_sb = big.tile([1, 2 * O], F32)
            # a = gamma*rstd ; b = beta - mean*a
            nc.vector.tensor_mul(ab_sb[:, 0:O], g_sb[:], rstd[:])
            nc.vector.tensor_mul(ab_sb[:, O:2 * O], mean[:], ab_sb[:, 0:O])
            nc.vector.tensor_sub(ab_sb[:, O:2 * O], be_sb[:],
                                 ab_sb[:, O:2 * O])
            nc.sync.dma_start(ab_d.ap().rearrange("(a b) -> a b", a=1),
                              ab_sb[:])
            ab_p = big.tile([2 * O, 1], F32)
            nc.sync.dma_start(ab_p[:],
                              ab_d.ap().rearrange("(a b) -> a b", b=1))

            # out = LeakyReLU(a*mfull + b) = max(a*m + b, alpha*(a*m + b)):
            # two scalar-engine affine passes + one DVE max
            ab2_p = big.tile([2 * O, 1], F32)
            nc.vector.tensor_scalar(out=ab2_p[:], in0=ab_p[:], scalar1=ALPHA,
                                    scalar2=None, op0=ALU.mult)
            obuf = big.tile([O, N], F32)
            obuf2 = ssb.tile([O, N], F32, tag="s")
            nc.scalar.activation(out=obuf[:], in_=mfull[:], func=AF.Identity,
                                 bias=ab_p[O:2 * O, :], scale=ab_p[0:O, :])
            nc.scalar.activation(out=obuf2[:], in_=mfull[:], func=AF.Identity,
                                 bias=ab2_p[O:2 * O, :], scale=ab2_p[0:O, :])
            nc.vector.tensor_tensor(out=obuf[:], in0=obuf[:], in1=obuf2[:],
                                    op=ALU.max)
            nc.sync.dma_start(out_d[:, :], obuf[:])

    nc.compile()
    return nc


def _get_nc():
    if "nc" not in _CACHED:
        _CACHED["nc"] = _build()
    return _CACHED["nc"]


def kernel(x, W, gamma, beta):
    from concourse.bass_utils import run_bass_kernel_spmd

    x = np.ascontiguousarray(np.asarray(x, dtype=np.float32))
    W = np.asarray(W, dtype=np.float32)
    gamma = np.asarray(gamma, dtype=np.float32)
    beta = np.asarray(beta, dtype=np.float32)

    w1t = np.ascontiguousarray(W[:, :C].T)     # [C, O]
    w2t = np.ascontiguousarray(W[:, C:].T)     # [C, O]
    ident = np.eye(128, dtype=np.float32)
    gam = np.ascontiguousarray(gamma[None, :])
    bet = np.ascontiguousarray(beta[None, :])

    in_maps = [dict(x=x[b], w1t=w1t, w2t=w2t, gamma=gam, beta=bet,
                    ident=ident) for b in range(B)]
    nc = _get_nc()
    res = run_bass_kernel_spmd(nc, in_maps, core_ids=list(range(8)))
    out = np.stack([np.asarray(res.results[b]["out"]) for b in range(B)])
    return out.astype(np.float32)


if __name__ == "__main__":
    rng = np.random.default_rng(0)
    x = rng.standard_normal((B, C, N)).astype(np.float32)
    W = (rng.standard_normal((O, 2 * C)) * 0.05).astype(np.float32)
    print(kernel(x, W, np.ones(O, np.float32), np.zeros(O, np.float32)).shape)


# revision 29
# speedup vs baseline: 1.0055x; 1.0055x over previous
"""EdgeConv (knn -> edge conv -> BN -> LeakyReLU -> max over k) on 8 NeuronCores.

Sharding: data-parallel over batch B=8, one sample per core. BN batch
statistics are all-reduced across the 8 cores on-device.

Math: with W = [W1 | W2] (acting on [nbr-ctr | ctr]), define
  u[m, :] = W1 @ x[:, m]          (projected neighbor part)
  v[n, :] = (W2 - W1) @ x[:, n]   (projected center part)
then y[n, k, :] = u[idx[n, k], :] + v[n, :].  BN scale is monotone, so
  out[:, n] = LeakyReLU(a * (max_k u[idx[n,k]] + v[n]) + b)
with a = gamma*rsqrt(var+eps), b = beta - mean*a.  Stats (mean/var over
(B, N, K)) come from per-n fp16 sums of gathered u (and squares / v-cross
terms), reduced across rows by a PE ones-matmul and all-reduced over the
batch.

Selection (exact, fp32): per 512-column window take the top-16 via
max8 / match_replace / max8 + two max_index passes (capacity 16 >= the
max top-20 concentration per window), merge the 128 candidates with three
max8/match_replace rounds marking the global top-20 as NEG, then extract
the winners' global column indices with a mask*(32768-idx) compaction:
three max8 rounds over the masked-index image recover the 20 indices
without any per-winner mask-reduce scans.

Gather: u stored fp16 in DRAM (halving gather bytes); 20 per-k indirect
DMAs per row-tile (HW applies per-partition offsets only for [128, 1]
offset APs).  Max/sum/sumsq trees over the 20 gathered pages run in fp16
on DVE at 2x; squares come from the scalar engine; the final
affine+LeakyReLU runs as two scalar-engine affine passes plus one DVE max
over the transposed result.
"""
import sys
for p in ("/opt/trn_rl_repo", "/root/.axon_site/_ro/trn_rl_repo"):
    if p not in sys.path:
        sys.path.insert(0, p)

import numpy as np

B, C, N, O, K = 8, 64, 4096, 64, 20
EPS = 1e-5
ALPHA = 0.2
T = N // 128          # 32 row-tiles
SEG = 512             # psum eviction chunk (one bank)
NW = N // SEG
WSEL = 512            # selection window: top-16 each (max conc. 10)
NWS = N // WSEL       # 8 windows -> 128 cands/row
NCAND = NWS * 16
QBASE = 32768.0
NEG = -3.0e38

_CACHED = {}


def _build(num_devices=8):
    import concourse.bass as bass
    import concourse.bacc as bacc
    import concourse.mybir as mybir
    from concourse.tile import TileContext

    F32 = mybir.dt.float32
    F16 = mybir.dt.float16
    U16 = mybir.dt.uint16
    U32 = mybir.dt.uint32
    AF = mybir.ActivationFunctionType
    ALU = mybir.AluOpType

    nc = bacc.Bacc("TRN2", target_bir_lowering=False, num_devices=num_devices)

    x_d = nc.dram_tensor("x", [C, N], F32, kind="ExternalInput")
    w1t_d = nc.dram_tensor("w1t", [C, O], F32, kind="ExternalInput")
    w2t_d = nc.dram_tensor("w2t", [C, O], F32, kind="ExternalInput")
    gam_d = nc.dram_tensor("gamma", [1, O], F32, kind="ExternalInput")
    bet_d = nc.dram_tensor("beta", [1, O], F32, kind="ExternalInput")
    id_d = nc.dram_tensor("ident", [128, 128], F32, kind="ExternalInput")
    out_d = nc.dram_tensor("out", [O, N], F32, kind="ExternalOutput")

    u_d = nc.dram_tensor("u_scratch", [N, O], F16)
    cc_in = nc.dram_tensor("cc_in", [5 * O], F32)
    cc_out = nc.dram_tensor("cc_out", [5 * O], F32)
    ab_d = nc.dram_tensor("ab_scratch", [2 * O], F32)

    CNT = float(B * N * K)

    with TileContext(nc) as tc:
        with tc.tile_pool(name="big", bufs=1) as big, \
             tc.tile_pool(name="sc", bufs=2) as sc, \
             tc.tile_pool(name="ssb", bufs=2) as ssb, \
             tc.tile_pool(name="gpool", bufs=3) as gpool, \
             tc.tile_pool(name="chup", bufs=3) as chup, \
             tc.tile_pool(name="ps", bufs=2, space="PSUM") as ps, \
             tc.tile_pool(name="pt", bufs=2, space="PSUM") as pt, \
             tc.tile_pool(name="pstat", bufs=1, space="PSUM") as pstat:

            # ---------------- phase 0: prep ----------------
            zmv = big.tile([C + 1, N], F32)      # moving: [x; -xx]
            zst = big.tile([C + 1, N], F32)      # stationary: [2x; ones]
            for cs in range(NW):
                nc.sync.dma_start(zmv[0:C, SEG * cs:SEG * (cs + 1)],
                                  x_d[:, SEG * cs:SEG * (cs + 1)])
            id_sb = big.tile([128, 128], F32)
            nc.sync.dma_start(id_sb[:], id_d[:, :])
            w1_sb = big.tile([C, O], F32)
            nc.sync.dma_start(w1_sb[:], w1t_d[:, :])
            w2_sb = big.tile([C, O], F32)
            nc.sync.dma_start(w2_sb[:], w2t_d[:, :])
            g_sb = big.tile([1, O], F32)
            nc.sync.dma_start(g_sb[:], gam_d[:, :])
            be_sb = big.tile([1, O], F32)
            nc.sync.dma_start(be_sb[:], bet_d[:, :])

            wv_sb = big.tile([C, O], F32)
            nc.vector.tensor_sub(wv_sb[:], w2_sb[:], w1_sb[:])

            for cs in range(NW):
                nc.vector.tensor_scalar(out=zst[0:C, SEG * cs:SEG * (cs + 1)],
                                        in0=zmv[0:C, SEG * cs:SEG * (cs + 1)],
                                        scalar1=2.0, scalar2=None, op0=ALU.mult)
            nc.vector.memset(zst[C:C + 1, :], 1.0)

            ones64 = big.tile([C, 1], F32)
            nc.vector.memset(ones64[:], 1.0)
            ones128 = big.tile([128, 1], F16)
            nc.vector.memset(ones128[:], 1.0)
            for cs in range(NW):
                sqx = sc.tile([C, SEG], F32, tag="sqx")
                nc.vector.tensor_mul(sqx[:], zmv[0:C, SEG * cs:SEG * (cs + 1)],
                                     zmv[0:C, SEG * cs:SEG * (cs + 1)])
                xx_ps = ps.tile([1, SEG], F32, tag="s")
                nc.tensor.matmul(xx_ps[:], ones64[:], sqx[:],
                                 start=True, stop=True)
                nc.scalar.activation(out=zmv[C:C + 1, SEG * cs:SEG * (cs + 1)],
                                     in_=xx_ps[:], func=AF.Copy, scale=-1.0)

            u16_sb = big.tile([128, T, O], F16)
            v32_sb = big.tile([128, T, O], F32)
            v16_sb = big.tile([128, T, O], F16)

            def emit_uv():
                # emitted after sel(0) so the 64 projection matmuls and the
                # u-table DRAM write hide behind the first selection pass
                for t in range(T):
                    up = ps.tile([128, O], F32, tag="uv")
                    nc.tensor.matmul(up[:], zmv[0:C, 128 * t:128 * (t + 1)],
                                     w1_sb[:], start=True, stop=True)
                    nc.scalar.activation(out=u16_sb[:, t, :], in_=up[:],
                                         func=AF.Copy)
                    vp = ps.tile([128, O], F32, tag="uv")
                    nc.tensor.matmul(vp[:], zmv[0:C, 128 * t:128 * (t + 1)],
                                     wv_sb[:], start=True, stop=True)
                    nc.scalar.activation(out=v32_sb[:, t, :], in_=vp[:],
                                         func=AF.Copy)
                    nc.scalar.activation(out=v16_sb[:, t, :], in_=vp[:],
                                         func=AF.Copy)
                nc.sync.dma_start(u_d.ap().rearrange("(t p) o -> p t o", p=128),
                                  u16_sb[:])

            # per-slot constant: QBASE - window_start(c), c -> (c // 16) * WSEL
            qoffs_c = big.tile([128, NCAND], F32)
            for w in range(NWS):
                nc.vector.memset(qoffs_c[:, 16 * w:16 * (w + 1)],
                                 QBASE - float(WSEL * w))
            mvp = big.tile([128, 8], F32)        # partial-round scratch
            nc.vector.memset(mvp[:], NEG)
            pcol_u = big.tile([128, 1], U32)     # partition index column
            nc.gpsimd.iota(pcol_u[:], pattern=[[0, 1]], base=0,
                           channel_multiplier=1)
            pcol = big.tile([128, 1], F32)
            nc.vector.tensor_copy(pcol[:], pcol_u[:])
            selfq = big.tile([128, 1], F32)      # per-tile QBASE-(128t+p)

            mfull = big.tile([O, N], F32)        # pre-norm max+v, transposed
            stats_ps = pstat.tile([1, 5 * O], F32)

            # ---------------- phase 1: per row-tile, software-pipelined ----
            # DVE emission order: sel(0), sel(1), trees(0), sel(2), trees(1),
            # ... so the k-gather of tile t completes behind selection of tile
            # t+1 instead of stalling the vector engine (~13us/tile).
            def emit_sel(t):
                s_sb = ssb.tile([128, N], F32, tag="s")
                s2_sb = ssb.tile([128, N], F32, tag="s2")
                for w in range(NW):
                    sp = ps.tile([128, SEG], F32, tag="s")
                    nc.tensor.matmul(sp[:], zst[:, 128 * t:128 * (t + 1)],
                                     zmv[:, SEG * w:SEG * (w + 1)],
                                     start=True, stop=True)
                    nc.scalar.activation(out=s_sb[:, SEG * w:SEG * (w + 1)],
                                         in_=sp[:], func=AF.Copy)

                cand = sc.tile([128, NWS, 2, 8], F32, tag="cand")
                cloc = sc.tile([128, NWS, 2, 8], U16, tag="cloc")
                for w in range(NWS):
                    sw = s_sb[:, WSEL * w:WSEL * (w + 1)]
                    s2w = s2_sb[:, WSEL * w:WSEL * (w + 1)]
                    nc.vector.max(out=cand[:, w, 0, :], in_=sw)
                    nc.vector.match_replace(out=s2w,
                                            in_to_replace=cand[:, w, 0, :],
                                            in_values=sw, imm_value=NEG)
                    nc.vector.max(out=cand[:, w, 1, :], in_=s2w)
                    nc.vector.max_index(out=cloc[:, w, 0, :],
                                        in_max=cand[:, w, 0, :], in_values=sw)
                    nc.vector.max_index(out=cloc[:, w, 1, :],
                                        in_max=cand[:, w, 1, :], in_values=s2w)

                # globf = QBASE - (window_start + cloc) in one fused op
                globf = sc.tile([128, NCAND], F32, tag="globf")
                nc.vector.scalar_tensor_tensor(out=globf[:], in0=cloc[:],
                                               scalar=-1.0, in1=qoffs_c[:],
                                               op0=ALU.mult, op1=ALU.add)

                mv = sc.tile([128, 8], F32, tag="mv")
                for r in range(3):
                    nc.vector.max(out=mv[:], in_=cand[:])
                    if r < 2:
                        nc.vector.match_replace(out=cand[:], in_to_replace=mv[:],
                                                in_values=cand[:], imm_value=NEG)
                nc.vector.tensor_copy(mvp[:, 0:4], mv[:, 0:4])
                nc.vector.match_replace(out=cand[:], in_to_replace=mvp[:],
                                        in_values=cand[:], imm_value=NEG)

                q = sc.tile([128, NCAND], F32, tag="q")
                nc.vector.scalar_tensor_tensor(out=q[:], in0=cand[:], scalar=NEG,
                                               in1=globf[:], op0=ALU.is_equal,
                                               op1=ALU.mult)
                # self (always rank 1) is served from SBUF, not gathered:
                # zero its q entry so compaction yields the other 19 indices
                nc.vector.tensor_scalar(out=selfq[:], in0=pcol[:], scalar1=-1.0,
                                        scalar2=QBASE - 128.0 * t,
                                        op0=ALU.mult, op1=ALU.add)
                nc.vector.scalar_tensor_tensor(out=q[:], in0=q[:],
                                               scalar=selfq[:, 0:1], in1=q[:],
                                               op0=ALU.not_equal, op1=ALU.mult)
                qsel = sc.tile([128, 24], F32, tag="qsel")
                for r in range(3):
                    nc.vector.max(out=qsel[:, 8 * r:8 * (r + 1)], in_=q[:])
                    if r < 2:
                        nc.vector.match_replace(out=q[:],
                                                in_to_replace=qsel[:, 8 * r:8 * (r + 1)],
                                                in_values=q[:], imm_value=0.0)
                nc.vector.tensor_scalar(out=qsel[:], in0=qsel[:],
                                        scalar1=-1.0, scalar2=QBASE,
                                        op0=ALU.mult, op1=ALU.add)
                chu = chup.tile([128, 24], U32, tag="chu")
                nc.vector.tensor_copy(chu[:], qsel[:])
                return chu

            def emit_gather(chu):
                gat = gpool.tile([128, K, O], F16, tag="gat")
                for k in range(K - 1):
                    nc.gpsimd.indirect_dma_start(
                        out=gat[:, k, :], out_offset=None, in_=u_d[:],
                        in_offset=bass.IndirectOffsetOnAxis(
                            ap=chu[:, k:k + 1], axis=0))
                return gat

            def emit_trees(t, gat):
                nc.scalar.activation(out=gat[:, K - 1, :], in_=u16_sb[:, t, :],
                                     func=AF.Copy)
                gv = gat[:, :, 0:O]
                gsq = gpool.tile([128, K, O], F16, tag="gsq")
                nc.scalar.activation(out=gsq[:], in_=gv, func=AF.Square)

                statcat = sc.tile([128, 3 * O], F16, tag="statcat")
                tm10 = sc.tile([128, 10, O], F16, tag="tm10")
                nc.vector.tensor_tensor(out=tm10[:], in0=gat[:, 0:10, 0:O],
                                        in1=gat[:, 10:20, 0:O], op=ALU.max)
                tm5 = sc.tile([128, 5, O], F16, tag="tm5")
                nc.vector.tensor_tensor(out=tm5[:], in0=tm10[:, 0:5, :],
                                        in1=tm10[:, 5:10, :], op=ALU.max)
                tm2 = sc.tile([128, 2, O], F16, tag="tm2")
                nc.vector.tensor_tensor(out=tm2[:], in0=tm5[:, 0:2, :],
                                        in1=tm5[:, 2:4, :], op=ALU.max)
                tm1 = sc.tile([128, O], F16, tag="tm1")
                nc.vector.tensor_tensor(out=tm1[:], in0=tm2[:, 0, :],
                                        in1=tm2[:, 1, :], op=ALU.max)
                mx = sc.tile([128, O], F16, tag="mx")
                nc.vector.tensor_tensor(out=mx[:], in0=tm1[:], in1=tm5[:, 4, :],
                                        op=ALU.max)

                ts10 = sc.tile([128, 10, O], F16, tag="ts10")
                nc.vector.tensor_tensor(out=ts10[:], in0=gat[:, 0:10, 0:O],
                                        in1=gat[:, 10:20, 0:O], op=ALU.add)
                ts5 = sc.tile([128, 5, O], F16, tag="ts5")
                nc.vector.tensor_tensor(out=ts5[:], in0=ts10[:, 0:5, :],
                                        in1=ts10[:, 5:10, :], op=ALU.add)
                ts2 = sc.tile([128, 2, O], F16, tag="ts2")
                nc.vector.tensor_tensor(out=ts2[:], in0=ts5[:, 0:2, :],
                                        in1=ts5[:, 2:4, :], op=ALU.add)
                ts1 = sc.tile([128, O], F16, tag="ts1")
                nc.vector.tensor_tensor(out=ts1[:], in0=ts2[:, 0, :],
                                        in1=ts2[:, 1, :], op=ALU.add)
                su = statcat[:, 0 * O:1 * O]
                nc.vector.tensor_tensor(out=su, in0=ts1[:], in1=ts5[:, 4, :],
                                        op=ALU.add)

                qs10 = sc.tile([128, 10, O], F16, tag="qs10")
                nc.vector.tensor_tensor(out=qs10[:], in0=gsq[:, 0:10, :],
                                        in1=gsq[:, 10:20, :], op=ALU.add)
                qs5 = sc.tile([128, 5, O], F16, tag="qs5")
                nc.vector.tensor_tensor(out=qs5[:], in0=qs10[:, 0:5, :],
                                        in1=qs10[:, 5:10, :], op=ALU.add)
                qs2 = sc.tile([128, 2, O], F16, tag="qs2")
                nc.vector.tensor_tensor(out=qs2[:], in0=qs5[:, 0:2, :],
                                        in1=qs5[:, 2:4, :], op=ALU.add)
                qs1 = sc.tile([128, O], F16, tag="qs1")
                nc.vector.tensor_tensor(out=qs1[:], in0=qs2[:, 0, :],
                                        in1=qs2[:, 1, :], op=ALU.add)
                ssq = statcat[:, 1 * O:2 * O]
                nc.vector.tensor_tensor(out=ssq, in0=qs1[:], in1=qs5[:, 4, :],
                                        op=ALU.add)
                vsu = statcat[:, 2 * O:3 * O]
                nc.vector.tensor_mul(vsu, v16_sb[:, t, :], su)

                st, sp_ = (t == 0), (t == T - 1)
                nc.tensor.matmul(stats_ps[:, 0:3 * O], ones128[:], statcat[:],
                                 start=st, stop=sp_, skip_group_check=True)

                mxv = sc.tile([128, O], F32, tag="mxv")
                nc.vector.tensor_tensor(out=mxv[:], in0=mx[:],
                                        in1=v32_sb[:, t, :], op=ALU.add)
                mt_ps = pt.tile([O, 128], F32, tag="mt")
                nc.tensor.transpose(out=mt_ps[:], in_=mxv[:], identity=id_sb[:])
                nc.scalar.activation(out=mfull[:, 128 * t:128 * (t + 1)],
                                     in_=mt_ps[:], func=AF.Copy)

            # depth-3 pipeline: trees(t) runs after sel(t+2), hiding both the
            # u-table warmup and per-tile gather latency
            gats = {}
            for t in range(T):
                chu = emit_sel(t)
                if t == 0:
                    emit_uv()   # u table lands while sel(0) runs on DVE
                gats[t] = emit_gather(chu)
                if t >= 2:
                    emit_trees(t - 2, gats.pop(t - 2))
            emit_trees(T - 2, gats.pop(T - 2))
            emit_trees(T - 1, gats.pop(T - 1))

            # v sums (once): Sv, Sv2 into stats_ps[:, 192:320]
            vsq16 = big.tile([128, T, O], F16)
            nc.scalar.activation(out=vsq16[:], in_=v16_sb[:], func=AF.Square)
            for t in range(T):
                st, sp_ = (t == 0), (t == T - 1)
                nc.tensor.matmul(stats_ps[:, 3 * O:4 * O], ones128[:],
                                 v16_sb[:, t, :], start=st, stop=sp_,
                                 skip_group_check=True)
                nc.tensor.matmul(stats_ps[:, 4 * O:5 * O], ones128[:],
                                 vsq16[:, t, :], start=st, stop=sp_,
                                 skip_group_check=True)

            # ---------------- phase 2: stats allreduce + finalize ----------
            stats_sb = big.tile([1, 5 * O], F32)
            nc.scalar.activation(out=stats_sb[:], in_=stats_ps[:], func=AF.Copy)
            if num_devices > 1:
                nc.sync.dma_start(cc_in.ap().rearrange("(a b) -> a b", a=1),
                                  stats_sb[:])
                nc.gpsimd.collective_compute(
                    "AllReduce", mybir.AluOpType.add,
                    replica_groups=[list(range(num_devices))],
                    ins=[cc_in.ap().opt()], outs=[cc_out.ap().opt()])
                sall = big.tile([1, 5 * O], F32)
                nc.sync.dma_start(sall[:],
                                  cc_out.ap().rearrange("(a b) -> a b", a=1))
            else:
                sall = stats_sb

            Sg = sall[:, 0 * O:1 * O]
            Sq = sall[:, 1 * O:2 * O]
            Svsu = sall[:, 2 * O:3 * O]
            Sv = sall[:, 3 * O:4 * O]
            Sv2 = sall[:, 4 * O:5 * O]

            mean = big.tile([1, O], F32)
            # mean = (Sg + K*Sv)/CNT
            nc.vector.tensor_scalar(out=mean[:], in0=Sv[:], scalar1=float(K),
                                    scalar2=None, op0=ALU.mult)
            nc.vector.tensor_add(mean[:], mean[:], Sg[:])
            nc.vector.tensor_scalar(out=mean[:], in0=mean[:],
                                    scalar1=1.0 / CNT, scalar2=None,
                                    op0=ALU.mult)
            ey2 = big.tile([1, O], F32)
            # ey2 = (Sq + 2*Svsu + K*Sv2)/CNT
            nc.vector.tensor_scalar(out=ey2[:], in0=Svsu[:], scalar1=2.0,
                                    scalar2=None, op0=ALU.mult)
            nc.vector.tensor_add(ey2[:], ey2[:], Sq[:])
            tmp = big.tile([1, O], F32)
            nc.vector.tensor_scalar(out=tmp[:], in0=Sv2[:], scalar1=float(K),
                                    scalar2=None, op0=ALU.mult)
            nc.vector.tensor_add(ey2[:], ey2[:], tmp[:])
            nc.vector.tensor_scalar(out=ey2[:], in0=ey2[:], scalar1=1.0 / CNT,
                                    scalar2=None, op0=ALU.mult)
            var = big.tile([1, O], F32)
            nc.vector.tensor_mul(var[:], mean[:], mean[:])
            nc.vector.tensor_sub(var[:], ey2[:], var[:])
            # rstd = 1/sqrt(var+eps) with two Newton-Raphson refinements
            std = big.tile([1, O], F32)
            epsb = big.tile([1, 1], F32)
            nc.vector.memset(epsb[:], EPS)
            ve = big.tile([1, O], F32)
            nc.vector.tensor_scalar(out=ve[:], in0=var[:], scalar1=EPS,
                                    scalar2=None, op0=ALU.add)
            nc.scalar.activation(out=std[:], in_=var[:], func=AF.Sqrt,
                                 bias=epsb[:], scale=1.0)
            rstd = big.tile([1, O], F32)
            nc.vector.reciprocal(rstd[:], std[:])
            nr1 = big.tile([1, O], F32)
            nr2 = big.tile([1, O], F32)
            for _ in range(2):
                nc.vector.tensor_mul(nr1[:], rstd[:], rstd[:])
                nc.vector.tensor_mul(nr1[:], nr1[:], ve[:])
                nc.vector.tensor_scalar(out=nr2[:], in0=nr1[:], scalar1=-0.5,
                                        scalar2=1.5, op0=ALU.mult, op1=ALU.add)
                nc.vector.tensor_mul(rstd[:], rstd[:], nr2[:])

            ab_sb = big.tile([1, 2 * O], F32)
            # a = gamma*rstd ; b = beta - mean*a
            nc.vector.tensor_mul(ab_sb[:, 0:O], g_sb[:], rstd[:])
            nc.vector.tensor_mul(ab_sb[:, O:2 * O], mean[:], ab_sb[:, 0:O])
            nc.vector.tensor_sub(ab_sb[:, O:2 * O], be_sb[:],
                                 ab_sb[:, O:2 * O])
            nc.sync.dma_start(ab_d.ap().rearrange("(a b) -> a b", a=1),
                              ab_sb[:])
            ab_p = big.tile([2 * O, 1], F32)
            nc.sync.dma_start(ab_p[:],
                              ab_d.ap().rearrange("(a b) -> a b", b=1))

            # out = LeakyReLU(a*mfull + b) = max(a*m + b, alpha*(a*m + b)):
            # two scalar-engine affine passes + one DVE max
            ab2_p = big.tile([2 * O, 1], F32)
            nc.vector.tensor_scalar(out=ab2_p[:], in0=ab_p[:], scalar1=ALPHA,
                                    scalar2=None, op0=ALU.mult)
            obuf = big.tile([O, N], F32)
            obuf2 = ssb.tile([O, N], F32, tag="s")
            nc.scalar.activation(out=obuf[:], in_=mfull[:], func=AF.Identity,
                                 bias=ab_p[O:2 * O, :], scale=ab_p[0:O, :])
            nc.scalar.activation(out=obuf2[:], in_=mfull[:], func=AF.Identity,
                                 bias=ab2_p[O:2 * O, :], scale=ab2_p[0:O, :])
            nc.vector.tensor_tensor(out=obuf[:], in0=obuf[:], in1=obuf2[:],
                                    op=ALU.max)
            nc.sync.dma_start(out_d[:, :], obuf[:])

    nc.compile()
    return nc


def _get_nc():
    if "nc" not in _CACHED:
        _CACHED["nc"] = _build()
    return _CACHED["nc"]


def kernel(x, W, gamma, beta):
    from concourse.bass_utils import run_bass_kernel_spmd

    x = np.ascontiguousarray(np.asarray(x, dtype=np.float32))
    W = np.asarray(W, dtype=np.float32)
    gamma = np.asarray(gamma, dtype=np.float32)
    beta = np.asarray(beta, dtype=np.float32)

    w1t = np.ascontiguousarray(W[:, :C].T)     # [C, O]
    w2t = np.ascontiguousarray(W[:, C:].T)     # [C, O]
    ident = np.eye(128, dtype=np.float32)
    gam = np.ascontiguousarray(gamma[None, :])
    bet = np.ascontiguousarray(beta[None, :])

    in_maps = [dict(x=x[b], w1t=w1t, w2t=w2t, gamma=gam, beta=bet,
                    ident=ident) for b in range(B)]
    nc = _get_nc()
    res = run_bass_kernel_spmd(nc, in_maps, core_ids=list(range(8)))
    out = np.stack([np.asarray(res.results[b]["out"]) for b in range(B)])
    return out.astype(np.float32)


if __name__ == "__main__":
    rng = np.random.default_rng(0)
    x = rng.standard_normal((B, C, N)).astype(np.float32)
    W = (rng.standard_normal((O, 2 * C)) * 0.05).astype(np.float32)
    print(kernel(x, W, np.ones(O, np.float32), np.zeros(O, np.float32)).shape)


# revision 30
# speedup vs baseline: 1.1951x; 1.1885x over previous
"""EdgeConv (knn -> edge conv -> BN -> LeakyReLU -> max over k) on 8 NeuronCores.

Sharding: data-parallel over batch B=8, one sample per core. BN batch
statistics are all-reduced across the 8 cores on-device.

Math: with W = [W1 | W2] (acting on [nbr-ctr | ctr]), define
  u[m, :] = W1 @ x[:, m]          (projected neighbor part)
  v[n, :] = (W2 - W1) @ x[:, n]   (projected center part)
then y[n, k, :] = u[idx[n, k], :] + v[n, :].  BN scale is monotone, so
  out[:, n] = LeakyReLU(a * (max_k u[idx[n,k]] + v[n]) + b)
with a = gamma*rsqrt(var+eps), b = beta - mean*a.  Stats (mean/var over
(B, N, K)) come from per-n fp16 sums of gathered u (and squares / v-cross
terms), reduced across rows by a PE ones-matmul and all-reduced over the
batch.

Selection (exact, fp32): per 512-column window take the top-16 via
max8 / match_replace / max8 + two max_index passes (capacity 16 >= the
max top-20 concentration per window), merge the 128 candidates with three
max8/match_replace rounds marking the global top-20 as NEG, then extract
the winners' global column indices with a mask*(32768-idx) compaction:
three max8 rounds over the masked-index image recover the 20 indices
without any per-winner mask-reduce scans.

Gather: u stored fp16 in DRAM (halving gather bytes); 19 per-k indirect
DMAs per row-tile (HW applies per-partition offsets only for [128, 1]
offset APs; the self neighbor, always rank 1, is copied from SBUF by the
scalar engine).  A depth-3 software pipeline emits trees(t) after
sel(t+2) so gather latency hides behind selection.  Max/sum/sumsq trees
over the 20 pages run in fp16 on DVE at 2x; squares come from the scalar
engine; the final affine+LeakyReLU runs as two scalar-engine affine
passes plus one DVE max over the transposed result.
"""
import sys
for p in ("/opt/trn_rl_repo", "/root/.axon_site/_ro/trn_rl_repo"):
    if p not in sys.path:
        sys.path.insert(0, p)

import numpy as np

B, C, N, O, K = 8, 64, 4096, 64, 20
EPS = 1e-5
ALPHA = 0.2
T = N // 128          # 32 row-tiles
SEG = 512             # psum eviction chunk (one bank)
NW = N // SEG
WSEL = 512            # selection window: top-16 each (max conc. 10)
NWS = N // WSEL       # 8 windows -> 128 cands/row
NCAND = NWS * 16
QBASE = 32768.0
NEG = -3.0e38

_CACHED = {}


def _build(num_devices=8):
    import concourse.bass as bass
    import concourse.bacc as bacc
    import concourse.mybir as mybir
    from concourse.tile import TileContext

    F32 = mybir.dt.float32
    F16 = mybir.dt.float16
    U16 = mybir.dt.uint16
    U32 = mybir.dt.uint32
    AF = mybir.ActivationFunctionType
    ALU = mybir.AluOpType

    nc = bacc.Bacc("TRN2", target_bir_lowering=False, num_devices=num_devices)

    x_d = nc.dram_tensor("x", [C, N], F32, kind="ExternalInput")
    w1t_d = nc.dram_tensor("w1t", [C, O], F32, kind="ExternalInput")
    w2t_d = nc.dram_tensor("w2t", [C, O], F32, kind="ExternalInput")
    gam_d = nc.dram_tensor("gamma", [1, O], F32, kind="ExternalInput")
    bet_d = nc.dram_tensor("beta", [1, O], F32, kind="ExternalInput")
    id_d = nc.dram_tensor("ident", [128, 128], F32, kind="ExternalInput")
    out_d = nc.dram_tensor("out", [O, N], F32, kind="ExternalOutput")

    u_d = nc.dram_tensor("u_scratch", [N, O], F16)
    cc_in = nc.dram_tensor("cc_in", [5 * O], F32)
    cc_out = nc.dram_tensor("cc_out", [5 * O], F32)
    ab_d = nc.dram_tensor("ab_scratch", [2 * O], F32)

    CNT = float(B * N * K)

    with TileContext(nc) as tc:
        with tc.tile_pool(name="big", bufs=1) as big, \
             tc.tile_pool(name="sc", bufs=2) as sc, \
             tc.tile_pool(name="ssb", bufs=2) as ssb, \
             tc.tile_pool(name="gpool", bufs=3) as gpool, \
             tc.tile_pool(name="chup", bufs=3) as chup, \
             tc.tile_pool(name="ps", bufs=2, space="PSUM") as ps, \
             tc.tile_pool(name="pt", bufs=2, space="PSUM") as pt, \
             tc.tile_pool(name="pstat", bufs=1, space="PSUM") as pstat:

            # ---------------- phase 0: prep ----------------
            zmv = big.tile([C + 1, N], F32)      # moving: [x; -xx]
            zst = big.tile([C + 1, N], F32)      # stationary: [2x; ones]
            for cs in range(NW):
                nc.sync.dma_start(zmv[0:C, SEG * cs:SEG * (cs + 1)],
                                  x_d[:, SEG * cs:SEG * (cs + 1)])
            id_sb = big.tile([128, 128], F32)
            nc.sync.dma_start(id_sb[:], id_d[:, :])
            w1_sb = big.tile([C, O], F32)
            nc.sync.dma_start(w1_sb[:], w1t_d[:, :])
            w2_sb = big.tile([C, O], F32)
            nc.sync.dma_start(w2_sb[:], w2t_d[:, :])
            g_sb = big.tile([1, O], F32)
            nc.sync.dma_start(g_sb[:], gam_d[:, :])
            be_sb = big.tile([1, O], F32)
            nc.sync.dma_start(be_sb[:], bet_d[:, :])

            wv_sb = big.tile([C, O], F32)
            nc.vector.tensor_sub(wv_sb[:], w2_sb[:], w1_sb[:])

            for cs in range(NW):
                nc.vector.tensor_scalar(out=zst[0:C, SEG * cs:SEG * (cs + 1)],
                                        in0=zmv[0:C, SEG * cs:SEG * (cs + 1)],
                                        scalar1=2.0, scalar2=None, op0=ALU.mult)
            nc.vector.memset(zst[C:C + 1, :], 1.0)

            ones64 = big.tile([C, 1], F32)
            nc.vector.memset(ones64[:], 1.0)
            ones128 = big.tile([128, 1], F16)
            nc.vector.memset(ones128[:], 1.0)
            for cs in range(NW):
                sqx = sc.tile([C, SEG], F32, tag="sqx")
                nc.vector.tensor_mul(sqx[:], zmv[0:C, SEG * cs:SEG * (cs + 1)],
                                     zmv[0:C, SEG * cs:SEG * (cs + 1)])
                xx_ps = ps.tile([1, SEG], F32, tag="s")
                nc.tensor.matmul(xx_ps[:], ones64[:], sqx[:],
                                 start=True, stop=True)
                nc.scalar.activation(out=zmv[C:C + 1, SEG * cs:SEG * (cs + 1)],
                                     in_=xx_ps[:], func=AF.Copy, scale=-1.0)

            u16_sb = big.tile([128, T, O], F16)
            v32_sb = big.tile([128, T, O], F32)
            v16_sb = big.tile([128, T, O], F16)

            def emit_uv():
                # emitted after sel(0) so the 64 projection matmuls and the
                # u-table DRAM write hide behind the first selection pass
                for t in range(T):
                    up = ps.tile([128, O], F32, tag="uv")
                    nc.tensor.matmul(up[:], zmv[0:C, 128 * t:128 * (t + 1)],
                                     w1_sb[:], start=True, stop=True)
                    nc.scalar.activation(out=u16_sb[:, t, :], in_=up[:],
                                         func=AF.Copy)
                    vp = ps.tile([128, O], F32, tag="uv")
                    nc.tensor.matmul(vp[:], zmv[0:C, 128 * t:128 * (t + 1)],
                                     wv_sb[:], start=True, stop=True)
                    nc.scalar.activation(out=v32_sb[:, t, :], in_=vp[:],
                                         func=AF.Copy)
                    nc.scalar.activation(out=v16_sb[:, t, :], in_=vp[:],
                                         func=AF.Copy)
                nc.sync.dma_start(u_d.ap().rearrange("(t p) o -> p t o", p=128),
                                  u16_sb[:])

            # per-slot constant: QBASE - window_start(c), c -> (c // 16) * WSEL
            qoffs_c = big.tile([128, NCAND], F32)
            for w in range(NWS):
                nc.vector.memset(qoffs_c[:, 16 * w:16 * (w + 1)],
                                 QBASE - float(WSEL * w))
            mvp = big.tile([128, 8], F32)        # partial-round scratch
            nc.vector.memset(mvp[:], NEG)
            pcol_u = big.tile([128, 1], U32)     # partition index column
            nc.gpsimd.iota(pcol_u[:], pattern=[[0, 1]], base=0,
                           channel_multiplier=1)
            pcol = big.tile([128, 1], F32)
            nc.vector.tensor_copy(pcol[:], pcol_u[:])
            selfq = big.tile([128, 1], F32)      # per-tile QBASE-(128t+p)

            mfull = big.tile([O, N], F32)        # pre-norm max+v, transposed
            stats_ps = pstat.tile([1, 5 * O], F32)

            # ---------------- phase 1: per row-tile, software-pipelined ----
            # DVE emission order: sel(0), sel(1), trees(0), sel(2), trees(1),
            # ... so the k-gather of tile t completes behind selection of tile
            # t+1 instead of stalling the vector engine (~13us/tile).
            def emit_sel(t):
                s_sb = ssb.tile([128, N], F32, tag="s")
                s2_sb = ssb.tile([128, N], F32, tag="s2")
                for w in range(NW):
                    sp = ps.tile([128, SEG], F32, tag="s")
                    nc.tensor.matmul(sp[:], zst[:, 128 * t:128 * (t + 1)],
                                     zmv[:, SEG * w:SEG * (w + 1)],
                                     start=True, stop=True)
                    nc.scalar.activation(out=s_sb[:, SEG * w:SEG * (w + 1)],
                                         in_=sp[:], func=AF.Copy)

                cand = sc.tile([128, NWS, 2, 8], F32, tag="cand")
                cloc = sc.tile([128, NWS, 2, 8], U16, tag="cloc")
                for w in range(NWS):
                    sw = s_sb[:, WSEL * w:WSEL * (w + 1)]
                    s2w = s2_sb[:, WSEL * w:WSEL * (w + 1)]
                    nc.vector.max(out=cand[:, w, 0, :], in_=sw)
                    nc.vector.match_replace(out=s2w,
                                            in_to_replace=cand[:, w, 0, :],
                                            in_values=sw, imm_value=NEG)
                    nc.vector.max(out=cand[:, w, 1, :], in_=s2w)
                    nc.vector.max_index(out=cloc[:, w, 0, :],
                                        in_max=cand[:, w, 0, :], in_values=sw)
                    nc.vector.max_index(out=cloc[:, w, 1, :],
                                        in_max=cand[:, w, 1, :], in_values=s2w)

                # globf = QBASE - (window_start + cloc) in one fused op
                globf = sc.tile([128, NCAND], F32, tag="globf")
                nc.vector.scalar_tensor_tensor(out=globf[:], in0=cloc[:],
                                               scalar=-1.0, in1=qoffs_c[:],
                                               op0=ALU.mult, op1=ALU.add)

                mv = sc.tile([128, 8], F32, tag="mv")
                for r in range(3):
                    nc.vector.max(out=mv[:], in_=cand[:])
                    if r < 2:
                        nc.vector.match_replace(out=cand[:], in_to_replace=mv[:],
                                                in_values=cand[:], imm_value=NEG)
                nc.vector.tensor_copy(mvp[:, 0:4], mv[:, 0:4])
                nc.vector.match_replace(out=cand[:], in_to_replace=mvp[:],
                                        in_values=cand[:], imm_value=NEG)

                q = sc.tile([128, NCAND], F32, tag="q")
                nc.vector.scalar_tensor_tensor(out=q[:], in0=cand[:], scalar=NEG,
                                               in1=globf[:], op0=ALU.is_equal,
                                               op1=ALU.mult)
                # self (always rank 1) is served from SBUF, not gathered:
                # zero its q entry so compaction yields the other 19 indices
                nc.vector.tensor_scalar(out=selfq[:], in0=pcol[:], scalar1=-1.0,
                                        scalar2=QBASE - 128.0 * t,
                                        op0=ALU.mult, op1=ALU.add)
                nc.vector.scalar_tensor_tensor(out=q[:], in0=q[:],
                                               scalar=selfq[:, 0:1], in1=q[:],
                                               op0=ALU.not_equal, op1=ALU.mult)
                qsel = sc.tile([128, 24], F32, tag="qsel")
                for r in range(3):
                    nc.vector.max(out=qsel[:, 8 * r:8 * (r + 1)], in_=q[:])
                    if r < 2:
                        nc.vector.match_replace(out=q[:],
                                                in_to_replace=qsel[:, 8 * r:8 * (r + 1)],
                                                in_values=q[:], imm_value=0.0)
                nc.vector.tensor_scalar(out=qsel[:], in0=qsel[:],
                                        scalar1=-1.0, scalar2=QBASE,
                                        op0=ALU.mult, op1=ALU.add)
                chu = chup.tile([128, 24], U32, tag="chu")
                nc.vector.tensor_copy(chu[:], qsel[:])
                return chu

            def emit_gather(chu):
                gat = gpool.tile([128, K, O], F16, tag="gat")
                for k in range(K - 1):
                    nc.gpsimd.indirect_dma_start(
                        out=gat[:, k, :], out_offset=None, in_=u_d[:],
                        in_offset=bass.IndirectOffsetOnAxis(
                            ap=chu[:, k:k + 1], axis=0))
                return gat

            def emit_trees(t, gat):
                nc.scalar.activation(out=gat[:, K - 1, :], in_=u16_sb[:, t, :],
                                     func=AF.Copy)
                gv = gat[:, :, 0:O]
                gsq = gpool.tile([128, K, O], F16, tag="gsq")
                nc.scalar.activation(out=gsq[:], in_=gv, func=AF.Square)

                statcat = sc.tile([128, 3 * O], F16, tag="statcat")
                tm10 = sc.tile([128, 10, O], F16, tag="tm10")
                nc.vector.tensor_tensor(out=tm10[:], in0=gat[:, 0:10, 0:O],
                                        in1=gat[:, 10:20, 0:O], op=ALU.max)
                tm5 = sc.tile([128, 5, O], F16, tag="tm5")
                nc.vector.tensor_tensor(out=tm5[:], in0=tm10[:, 0:5, :],
                                        in1=tm10[:, 5:10, :], op=ALU.max)
                tm2 = sc.tile([128, 2, O], F16, tag="tm2")
                nc.vector.tensor_tensor(out=tm2[:], in0=tm5[:, 0:2, :],
                                        in1=tm5[:, 2:4, :], op=ALU.max)
                tm1 = sc.tile([128, O], F16, tag="tm1")
                nc.vector.tensor_tensor(out=tm1[:], in0=tm2[:, 0, :],
                                        in1=tm2[:, 1, :], op=ALU.max)
                mx = sc.tile([128, O], F16, tag="mx")
                nc.vector.tensor_tensor(out=mx[:], in0=tm1[:], in1=tm5[:, 4, :],
                                        op=ALU.max)

                ts10 = sc.tile([128, 10, O], F16, tag="ts10")
                nc.vector.tensor_tensor(out=ts10[:], in0=gat[:, 0:10, 0:O],
                                        in1=gat[:, 10:20, 0:O], op=ALU.add)
                ts5 = sc.tile([128, 5, O], F16, tag="ts5")
                nc.vector.tensor_tensor(out=ts5[:], in0=ts10[:, 0:5, :],
                                        in1=ts10[:, 5:10, :], op=ALU.add)
                ts2 = sc.tile([128, 2, O], F16, tag="ts2")
                nc.vector.tensor_tensor(out=ts2[:], in0=ts5[:, 0:2, :],
                                        in1=ts5[:, 2:4, :], op=ALU.add)
                ts1 = sc.tile([128, O], F16, tag="ts1")
                nc.vector.tensor_tensor(out=ts1[:], in0=ts2[:, 0, :],
                                        in1=ts2[:, 1, :], op=ALU.add)
                su = statcat[:, 0 * O:1 * O]
                nc.vector.tensor_tensor(out=su, in0=ts1[:], in1=ts5[:, 4, :],
                                        op=ALU.add)

                qs10 = sc.tile([128, 10, O], F16, tag="qs10")
                nc.vector.tensor_tensor(out=qs10[:], in0=gsq[:, 0:10, :],
                                        in1=gsq[:, 10:20, :], op=ALU.add)
                qs5 = sc.tile([128, 5, O], F16, tag="qs5")
                nc.vector.tensor_tensor(out=qs5[:], in0=qs10[:, 0:5, :],
                                        in1=qs10[:, 5:10, :], op=ALU.add)
                qs2 = sc.tile([128, 2, O], F16, tag="qs2")
                nc.vector.tensor_tensor(out=qs2[:], in0=qs5[:, 0:2, :],
                                        in1=qs5[:, 2:4, :], op=ALU.add)
                qs1 = sc.tile([128, O], F16, tag="qs1")
                nc.vector.tensor_tensor(out=qs1[:], in0=qs2[:, 0, :],
                                        in1=qs2[:, 1, :], op=ALU.add)
                ssq = statcat[:, 1 * O:2 * O]
                nc.vector.tensor_tensor(out=ssq, in0=qs1[:], in1=qs5[:, 4, :],
                                        op=ALU.add)
                vsu = statcat[:, 2 * O:3 * O]
                nc.vector.tensor_mul(vsu, v16_sb[:, t, :], su)

                st, sp_ = (t == 0), (t == T - 1)
                nc.tensor.matmul(stats_ps[:, 0:3 * O], ones128[:], statcat[:],
                                 start=st, stop=sp_, skip_group_check=True)

                mxv = sc.tile([128, O], F32, tag="mxv")
                nc.vector.tensor_tensor(out=mxv[:], in0=mx[:],
                                        in1=v32_sb[:, t, :], op=ALU.add)
                mt_ps = pt.tile([O, 128], F32, tag="mt")
                nc.tensor.transpose(out=mt_ps[:], in_=mxv[:], identity=id_sb[:])
                nc.scalar.activation(out=mfull[:, 128 * t:128 * (t + 1)],
                                     in_=mt_ps[:], func=AF.Copy)

            # depth-3 pipeline: trees(t) runs after sel(t+2), hiding both the
            # u-table warmup and per-tile gather latency
            gats = {}
            for t in range(T):
                chu = emit_sel(t)
                if t == 0:
                    emit_uv()   # u table lands while sel(0) runs on DVE
                gats[t] = emit_gather(chu)
                if t >= 2:
                    emit_trees(t - 2, gats.pop(t - 2))
            emit_trees(T - 2, gats.pop(T - 2))
            emit_trees(T - 1, gats.pop(T - 1))

            # v sums (once): Sv, Sv2 into stats_ps[:, 192:320]
            vsq16 = big.tile([128, T, O], F16)
            nc.scalar.activation(out=vsq16[:], in_=v16_sb[:], func=AF.Square)
            for t in range(T):
                st, sp_ = (t == 0), (t == T - 1)
                nc.tensor.matmul(stats_ps[:, 3 * O:4 * O], ones128[:],
                                 v16_sb[:, t, :], start=st, stop=sp_,
                                 skip_group_check=True)
                nc.tensor.matmul(stats_ps[:, 4 * O:5 * O], ones128[:],
                                 vsq16[:, t, :], start=st, stop=sp_,
                                 skip_group_check=True)

            # ---------------- phase 2: stats allreduce + finalize ----------
            stats_sb = big.tile([1, 5 * O], F32)
            nc.scalar.activation(out=stats_sb[:], in_=stats_ps[:], func=AF.Copy)
            if num_devices > 1:
                nc.sync.dma_start(cc_in.ap().rearrange("(a b) -> a b", a=1),
                                  stats_sb[:])
                nc.gpsimd.collective_compute(
                    "AllReduce", mybir.AluOpType.add,
                    replica_groups=[list(range(num_devices))],
                    ins=[cc_in.ap().opt()], outs=[cc_out.ap().opt()])
                sall = big.tile([1, 5 * O], F32)
                nc.sync.dma_start(sall[:],
                                  cc_out.ap().rearrange("(a b) -> a b", a=1))
            else:
                sall = stats_sb

            Sg = sall[:, 0 * O:1 * O]
            Sq = sall[:, 1 * O:2 * O]
            Svsu = sall[:, 2 * O:3 * O]
            Sv = sall[:, 3 * O:4 * O]
            Sv2 = sall[:, 4 * O:5 * O]

            mean = big.tile([1, O], F32)
            # mean = (Sg + K*Sv)/CNT
            nc.vector.tensor_scalar(out=mean[:], in0=Sv[:], scalar1=float(K),
                                    scalar2=None, op0=ALU.mult)
            nc.vector.tensor_add(mean[:], mean[:], Sg[:])
            nc.vector.tensor_scalar(out=mean[:], in0=mean[:],
                                    scalar1=1.0 / CNT, scalar2=None,
                                    op0=ALU.mult)
            ey2 = big.tile([1, O], F32)
            # ey2 = (Sq + 2*Svsu + K*Sv2)/CNT
            nc.vector.tensor_scalar(out=ey2[:], in0=Svsu[:], scalar1=2.0,
                                    scalar2=None, op0=ALU.mult)
            nc.vector.tensor_add(ey2[:], ey2[:], Sq[:])
            tmp = big.tile([1, O], F32)
            nc.vector.tensor_scalar(out=tmp[:], in0=Sv2[:], scalar1=float(K),
                                    scalar2=None, op0=ALU.mult)
            nc.vector.tensor_add(ey2[:], ey2[:], tmp[:])
            nc.vector.tensor_scalar(out=ey2[:], in0=ey2[:], scalar1=1.0 / CNT,
                                    scalar2=None, op0=ALU.mult)
            var = big.tile([1, O], F32)
            nc.vector.tensor_mul(var[:], mean[:], mean[:])
            nc.vector.tensor_sub(var[:], ey2[:], var[:])
            # rstd = 1/sqrt(var+eps) with two Newton-Raphson refinements
            std = big.tile([1, O], F32)
            epsb = big.tile([1, 1], F32)
            nc.vector.memset(epsb[:], EPS)
            ve = big.tile([1, O], F32)
            nc.vector.tensor_scalar(out=ve[:], in0=var[:], scalar1=EPS,
                                    scalar2=None, op0=ALU.add)
            nc.scalar.activation(out=std[:], in_=var[:], func=AF.Sqrt,
                                 bias=epsb[:], scale=1.0)
            rstd = big.tile([1, O], F32)
            nc.vector.reciprocal(rstd[:], std[:])
            nr1 = big.tile([1, O], F32)
            nr2 = big.tile([1, O], F32)
            for _ in range(2):
                nc.vector.tensor_mul(nr1[:], rstd[:], rstd[:])
                nc.vector.tensor_mul(nr1[:], nr1[:], ve[:])
                nc.vector.tensor_scalar(out=nr2[:], in0=nr1[:], scalar1=-0.5,
                                        scalar2=1.5, op0=ALU.mult, op1=ALU.add)
                nc.vector.tensor_mul(rstd[:], rstd[:], nr2[:])

            ab_sb = big.tile([1, 2 * O], F32)
            # a = gamma*rstd ; b = beta - mean*a
            nc.vector.tensor_mul(ab_sb[:, 0:O], g_sb[:], rstd[:])
            nc.vector.tensor_mul(ab_sb[:, O:2 * O], mean[:], ab_sb[:, 0:O])
            nc.vector.tensor_sub(ab_sb[:, O:2 * O], be_sb[:],
                                 ab_sb[:, O:2 * O])
            nc.sync.dma_start(ab_d.ap().rearrange("(a b) -> a b", a=1),
                              ab_sb[:])
            ab_p = big.tile([2 * O, 1], F32)
            nc.sync.dma_start(ab_p[:],
                              ab_d.ap().rearrange("(a b) -> a b", b=1))

            # out = LeakyReLU(a*mfull + b) = max(a*m + b, alpha*(a*m + b)):
            # two scalar-engine affine passes + one DVE max
            ab2_p = big.tile([2 * O, 1], F32)
            nc.vector.tensor_scalar(out=ab2_p[:], in0=ab_p[:], scalar1=ALPHA,
                                    scalar2=None, op0=ALU.mult)
            obuf = big.tile([O, N], F32)
            obuf2 = ssb.tile([O, N], F32, tag="s")
            nc.scalar.activation(out=obuf[:], in_=mfull[:], func=AF.Identity,
                                 bias=ab_p[O:2 * O, :], scale=ab_p[0:O, :])
            nc.scalar.activation(out=obuf2[:], in_=mfull[:], func=AF.Identity,
                                 bias=ab2_p[O:2 * O, :], scale=ab2_p[0:O, :])
            nc.vector.tensor_tensor(out=obuf[:], in0=obuf[:], in1=obuf2[:],
                                    op=ALU.max)
            nc.sync.dma_start(out_d[:, :], obuf[:])

    nc.compile()
    return nc


def _get_nc():
    if "nc" not in _CACHED:
        _CACHED["nc"] = _build()
    return _CACHED["nc"]


def kernel(x, W, gamma, beta):
    from concourse.bass_utils import run_bass_kernel_spmd

    x = np.ascontiguousarray(np.asarray(x, dtype=np.float32))
    W = np.asarray(W, dtype=np.float32)
    gamma = np.asarray(gamma, dtype=np.float32)
    beta = np.asarray(beta, dtype=np.float32)

    w1t = np.ascontiguousarray(W[:, :C].T)     # [C, O]
    w2t = np.ascontiguousarray(W[:, C:].T)     # [C, O]
    ident = np.eye(128, dtype=np.float32)
    gam = np.ascontiguousarray(gamma[None, :])
    bet = np.ascontiguousarray(beta[None, :])

    in_maps = [dict(x=x[b], w1t=w1t, w2t=w2t, gamma=gam, beta=bet,
                    ident=ident) for b in range(B)]
    nc = _get_nc()
    res = run_bass_kernel_spmd(nc, in_maps, core_ids=list(range(8)))
    out = np.stack([np.asarray(res.results[b]["out"]) for b in range(B)])
    return out.astype(np.float32)


if __name__ == "__main__":
    rng = np.random.default_rng(0)
    x = rng.standard_normal((B, C, N)).astype(np.float32)
    W = (rng.standard_normal((O, 2 * C)) * 0.05).astype(np.float32)
    print(kernel(x, W, np.ones(O, np.float32), np.zeros(O, np.float32)).shape)
